# revision 8
# baseline (speedup 1.0000x reference)
"""MultiHeadDoubleAttention on TRN2 — v4 (fp8 DoubleRow convs, interleaved
attention).

Data-parallel over batch: 8 cores x 16 batch each.

Conv: 15x15 conv with pad 7 on an 8x8 grid == sum over 65 non-masked taps
of channel-matmuls on shifted pixel rectangles.  fp8-e4m3 DoubleRow
matmuls contract all 256 input channels per instruction (2x bf16 rate).
Weights are scaled by 256 into fp8 range; activations un-scale.
Redundant LDWEIGHTS are deduped post-emission.

Conv passes are split per output-channel-half (oh) into 2-bank psum
sub-passes, ping-ponging two tags.  Attention for heads 0-3 is emitted
between P3's oh=0 and oh=1 halves so its vector/scalar work hides under
conv matmuls; only heads 4-7 and the output projection trail the convs.
"""
import os
import sys
sys.path.insert(0, '/opt/trn_rl_repo')
import numpy as np
import ml_dtypes

import concourse.bass as bass
import concourse.bacc as bacc
import concourse.mybir as mybir
import concourse.bass_utils as bass_utils
from concourse.tile import TileContext

F32 = mybir.dt.float32
BF16 = mybir.dt.bfloat16
FP8 = mybir.dt.float8e4
E4M3 = ml_dtypes.float8_e4m3
DR = mybir.MatmulPerfMode.DoubleRow

B, D, H, DK = 128, 256, 8, 32
NCORES = 8
BL = B // NCORES          # batch per core
NPIX = 64                 # 8x8
RS = 1.0 / np.sqrt(DK)    # score scale
WSCALE = 256.0            # fp8 weight scale
DR_MAXN = int(os.environ.get("DR_MAXN", "512"))


def hollow_mask():
    m = np.ones((15, 15), np.float32)
    for c in range(5):
        m[1 + c:7, c] = 0; m[8:14 - c, c] = 0
        m[c, 1 + c:7] = 0; m[c, 8:14 - c] = 0
        m[1 + c:7, 14 - c] = 0; m[8:14 - c, 14 - c] = 0
        m[14 - c, 1 + c:7] = 0; m[14 - c, 8:14 - c] = 0
    return m


def tap_schedule():
    m = hollow_mask()
    taps = []
    for di in range(15):
        for dj in range(15):
            if not m[di, dj]:
                continue
            sr, sc = di - 7, dj - 7
            ar0, ar1 = max(0, sr), min(7, 7 + sr)
            ac0, ac1 = max(0, sc), min(7, 7 + sc)
            taps.append((sr, sc, ar0, ar1 - ar0 + 1, ac0, ac1 - ac0 + 1))
    taps.sort(key=lambda e: -(e[3] * e[5]))
    return taps


TAPS = tap_schedule()
NT = len(TAPS)            # 65


def tap_pieces(sr, sc, ar0, hr, ac0, wc):
    """Split output rect at the ir=4 psum-bank boundary, cap moving dim."""
    ir0 = ar0 - sr
    ic0 = ac0 - sc
    raw = []
    lo, hi = ir0, ir0 + hr
    if lo < 4:
        ph = min(hi, 4) - lo
        raw.append((0, lo, lo + sr, ph, ic0, ac0, wc))
    if hi > 4:
        p0 = max(lo, 4)
        ph = hi - p0
        raw.append((1, p0 - 4, p0 + sr, ph, ic0, ac0, wc))
    pieces = []
    for (bk, irb, a0, ph, ic0_, ac0_, wc_) in raw:
        max_ph = max(1, DR_MAXN // (BL * wc_))
        r = 0
        while r < ph:
            n = min(max_ph, ph - r)
            pieces.append((bk, irb + r, a0 + r, n, ic0_, ac0_, wc_))
            r += n
    return pieces


PIECES = [tap_pieces(*t) for t in TAPS]
TOTAL_PIECES = {0: sum(1 for ps in PIECES for p in ps if p[0] == 0),
                1: sum(1 for ps in PIECES for p in ps if p[0] == 1)}


# ---------------------------------------------------------------------------
# LDWEIGHTS dedupe
# ---------------------------------------------------------------------------
def _ldw_sig(inst):
    ap = inst.ins[0]
    return (str(ap), str(inst.tile_position), str(inst.tile_size),
            str(inst.is_transpose), str(inst.perf_mode))


def dedupe_ldweights(nc):
    PE = mybir.EngineType.PE
    removed = 0
    for fn in nc.m.functions:
        for blk in fn.blocks:
            insts = list(blk.instructions)
            last_sig = None
            keep = []
            pending_waits = []
            for inst in insts:
                nm = type(inst).__name__
                if getattr(inst, 'engine', None) == PE:
                    if nm == 'InstLdweights':
                        sig = _ldw_sig(inst)
                        si = inst.sync_info
                        has_upd = si is not None and len(si.on_update) > 0
                        if sig == last_sig and not has_upd:
                            if si is not None and len(si.on_wait) > 0:
                                pending_waits.extend(si.on_wait)
                            removed += 1
                            continue
                        last_sig = sig
                    elif nm == 'InstMatmult':
                        pass
                    else:
                        last_sig = None
                    if pending_waits:
                        si = inst.sync_info
                        if si is None:
                            inst.sync_info = mybir.SyncInfo(
                                on_wait=list(pending_waits), on_update=[])
                        else:
                            si.on_wait = list(si.on_wait) + pending_waits
                        pending_waits = []
                keep.append(inst)
            assert not pending_waits
            if len(keep) != len(insts):
                blk.instructions.clear()
                for inst in keep:
                    blk.instructions.append(inst)
    return removed


# ---------------------------------------------------------------------------
# Conv sub-pass (fp8 DoubleRow, one oh half)
# ---------------------------------------------------------------------------
class WeightSet:
    def __init__(self, chunks):
        self.chunks = chunks      # list of (tap0, ntap, tile-slice)


def load_weights(nc, pool, w_dram, tag, resident, chunk=8):
    """w_dram: [128, NT, 2, 256] fp8; per-partition line contiguous."""
    chunks = []
    if resident:
        wt = pool.tile([128, NT, 2, 256], FP8, tag=f"{tag}wR", name=f"{tag}wR")
        c0 = 0
        while c0 < NT:
            n = min(2 if c0 == 0 else chunk, NT - c0)
            nc.sync.dma_start(wt[:, c0:c0 + n], w_dram[:, c0:c0 + n])
            chunks.append((c0, n, wt[:, c0:c0 + n]))
            c0 += n
    else:
        c0 = 0
        while c0 < NT:
            n = min(2 if c0 == 0 else chunk, NT - c0)
            wt = pool.tile([128, chunk, 2, 256], FP8, tag=f"{tag}wS",
                           name=f"{tag}wS{c0}")
            nc.sync.dma_start(wt[:, :n], w_dram[:, c0:c0 + n])
            chunks.append((c0, n, wt[:, :n]))
            c0 += n
    return WeightSet(chunks)


def conv_oh(nc, psum_pool, ws: WeightSet, inputs, oh, tag, ptags):
    """One oh half over all taps.  inputs: list of [128,2,8,8,BL] fp8 tiles.
    ptags: psum tag per input (2 banks each).  Returns ps[ii][bk]."""
    ps = [[psum_pool.tile([128, 4, 8, BL], F32, tag=f"{ptags[ii]}{bk}",
                          name=f"{tag}ps{ii}{bk}") for bk in range(2)]
          for ii in range(len(inputs))]
    done = {}
    for c0, n, wt in ws.chunks:
        for tl in range(n):
            pieces = PIECES[c0 + tl]
            lhsT = wt[:, tl, :, oh * 128:(oh + 1) * 128]
            for ii, xt in enumerate(inputs):
                for (bk, irb, ar0, ph, ic0, ac0, wc) in pieces:
                    key = (ii, bk)
                    cnt = done.get(key, 0)
                    done[key] = cnt + 1
                    rhs = xt[:, :, ar0:ar0 + ph, ac0:ac0 + wc, :]
                    out = ps[ii][bk][:, irb:irb + ph, ic0:ic0 + wc, :]
                    nc.tensor.matmul(out, lhsT, rhs,
                                     start=(cnt == 0),
                                     stop=(cnt == TOTAL_PIECES[bk] - 1),
                                     perf_mode=DR)
    return ps


def build_kernel():
    nc = bacc.Bacc("TRN2", target_bir_lowering=False, debug=False,
                   num_devices=NCORES)
    dt = {}
    for nmm in ("q", "k", "v"):
        dt[f"x{nmm}"] = nc.dram_tensor(f"x{nmm}", [128, 2, 8, 8, BL], FP8,
                                       kind="ExternalInput")
        dt[f"w{nmm}"] = nc.dram_tensor(f"w{nmm}", [128, NT, 2, 256], FP8,
                                       kind="ExternalInput")
        dt[f"bias{nmm}"] = nc.dram_tensor(f"bias{nmm}", [2, 128], F32,
                                          kind="ExternalInput")
    dt["wo_t"] = nc.dram_tensor("wo_t", [2, 128, 256], BF16, kind="ExternalInput")
    dt["bo"] = nc.dram_tensor("bo", [1, 256], BF16, kind="ExternalInput")
    dt["ones"] = nc.dram_tensor("ones", [1, 128], BF16, kind="ExternalInput")
    dt["ident"] = nc.dram_tensor("ident", [128, 128], BF16, kind="ExternalInput")
    dt["out"] = nc.dram_tensor("out", [8, 128, 256], F32, kind="ExternalOutput")

    with TileContext(nc) as tc:
      with tc.tile_pool(name="persist", bufs=1) as pp:
        bias_t = {}
        for nmm in ("q", "k", "v"):
            bias_t[nmm] = pp.tile([128, 2], F32, name=f"bias{nmm}_t")
        ones_t = pp.tile([1, 128], BF16, name="ones_t")
        bo_t = pp.tile([1, 256], BF16, name="bo_t")
        ident_t = pp.tile([128, 128], BF16, name="ident_t")
        wo_tt = [pp.tile([128, 256], BF16, name=f"wo_tt{h}") for h in range(2)]

        # conv2 outputs [o, b, pix] -- live into attention
        hh = {}
        for nmm in ("q", "k", "v"):
            hh[nmm] = [pp.tile([128, BL, NPIX], BF16, name=f"h{nmm}{h}")
                       for h in range(2)]
        # attention sbuf tiles
        E_t = pp.tile([128, BL // 2, H, NPIX], BF16, name="E_t")
        VT = pp.tile([128, BL // 2, H, 33], BF16, name="VT")
        OA = pp.tile([64, BL, 2, 128], BF16, name="OA")
        concat = pp.tile([128, 2, BL, NPIX], BF16, name="concat")
        rcp = pp.tile([64, BL, H], F32, name="rcp")
        out_sb = pp.tile([128, 8, 256], F32, name="out_sb")

        def emit_persist_dmas():
            for nmm in ("q", "k", "v"):
                nc.sync.dma_start(bias_t[nmm][:],
                                  dt[f"bias{nmm}"].ap().rearrange("h c -> c h"))
            nc.sync.dma_start(ones_t[:], dt["ones"][:])
            nc.sync.dma_start(bo_t[:], dt["bo"][:])
            nc.sync.dma_start(ident_t[:], dt["ident"][:])
            for h in range(2):
                nc.sync.dma_start(wo_tt[h][:], dt["wo_t"][h])

        def load_x(pool, nmm):
            xt = pool.tile([128, 2, 8, 8, BL], FP8, name=f"x{nmm}t")
            nc.sync.dma_start(xt[:], dt[f"x{nmm}"][:])
            return xt

        def relu_out(ps2, x1t, oh, bias):
            # ps2: [bk] psum pair for this oh; x1t [128,2,8,8,BL] fp8
            for bk in range(2):
                nc.scalar.activation(
                    x1t[:, oh, bk * 4:(bk + 1) * 4, :, :],
                    ps2[bk][:], mybir.ActivationFunctionType.Relu,
                    bias=bias[:, oh:oh + 1], scale=1.0 / WSCALE)

        def final_out(ps2, out_t, oh, bias):
            for bk in range(2):
                nc.scalar.activation(
                    out_t[oh][:, :, bk * 32:(bk + 1) * 32],
                    ps2[bk][:].rearrange("c pr pc b -> c b (pr pc)"),
                    mybir.ActivationFunctionType.Identity,
                    bias=bias[:, oh:oh + 1], scale=1.0 / WSCALE)

        def attn_half(atp, oh):
            """Scores, softmax, attn@V for heads oh*4..oh*4+3."""
            kh, qh = hh['k'], hh['q']
            for hp in range(4):
                h = oh * 4 + hp
                for b2p in range(BL // 4):
                    pst = atp.tile([128, 2, 64], F32, tag="pst",
                                   name=f"pst{h}{b2p}", bufs=2)
                    for j in range(2):
                        b2 = 2 * b2p + j
                        for par in range(2):
                            b = 2 * b2 + par
                            nc.tensor.matmul(
                                pst[64 * par:64 * par + 64, j, :],
                                kh[oh][hp * 32:(hp + 1) * 32, b, :],
                                qh[oh][hp * 32:(hp + 1) * 32, b, :],
                                start=True, stop=True,
                                tile_position=(32 * hp, 64 * par))
                    nc.scalar.activation(
                        E_t[:, 2 * b2p:2 * b2p + 2, h, :], pst[:],
                        mybir.ActivationFunctionType.Exp, scale=RS)
            for b in range(BL):
                par, b2 = b % 2, b // 2
                pso = atp.tile([64, 4, 33], F32, tag="pso",
                               name=f"pso{oh}{b}", bufs=2)
                for hp in range(4):
                    h = oh * 4 + hp
                    nc.tensor.matmul(pso[:, hp, :],
                                     E_t[64 * par:64 * par + 64, b2, h, :],
                                     VT[64 * par:64 * par + 64, b2, h, :],
                                     start=True, stop=True)
                nc.vector.reciprocal(rcp[:, b, oh * 4:oh * 4 + 4],
                                     pso[:, :, 32:33])
                nc.vector.tensor_tensor(
                    OA[:, b, oh].rearrange("k (h d) -> k h d", d=32),
                    pso[:, :, 0:32],
                    rcp[:, b, oh * 4:oh * 4 + 4, None].to_broadcast((64, 4, 32)),
                    mybir.AluOpType.mult)

        with tc.tile_pool(name="pq", bufs=1) as pqp, \
             tc.tile_pool(name="wstream", bufs=3) as wsp, \
             tc.tile_pool(name="pv", bufs=1) as pvp, \
             tc.tile_pool(name="pk", bufs=1) as pkp:
            with tc.tile_pool(name="cvps", bufs=1, space="PSUM") as psp:
                # ---- P1: q -> q1 (stream wq) ----
                xq = load_x(pqp, 'q')
                x1q = pqp.tile([128, 2, 8, 8, BL], FP8, name="x1q")
                wq = load_weights(nc, wsp, dt["wq"].ap(), tag="q1",
                                  resident=False)
                emit_persist_dmas()
                for oh in range(2):
                    ps = conv_oh(nc, psp, wq, [xq], oh, f"p1{oh}",
                                 [f"cvS{oh}"])
                    relu_out(ps[0], x1q, oh, bias_t['q'])

                # ---- P4/P5: v -> v1 -> vh (wv resident, prefetched) ----
                xv = load_x(pvp, 'v')
                x1v = pvp.tile([128, 2, 8, 8, BL], FP8, name="x1v")
                wv = load_weights(nc, pvp, dt["wv"].ap(), tag="v",
                                  resident=True)
                for oh in range(2):
                    ps = conv_oh(nc, psp, wv, [xv], oh, f"p4{oh}",
                                 [f"cvS{oh}"])
                    relu_out(ps[0], x1v, oh, bias_t['v'])
                for oh in range(2):
                    ps = conv_oh(nc, psp, wv, [x1v], oh, f"p5{oh}",
                                 [f"cvS{oh}"])
                    final_out(ps[0], hh['v'], oh, bias_t['v'])

                # ---- V transposes (hide under P2) ----
                nc.vector.memset(VT[:, :, :, 32:33], 1.0)
                with tc.tile_pool(name="vtps", bufs=1, space="PSUM") as vtp:
                    for b in range(BL):
                        par, b2 = b % 2, b // 2
                        pvt = vtp.tile([64, 2, 128], BF16, tag="pvt",
                                       name=f"pvt{b}", bufs=2)
                        for oh in range(2):
                            nc.tensor.transpose(pvt[:, oh, :],
                                                hh['v'][oh][:, b, :],
                                                ident_t[:])
                        nc.vector.tensor_copy(
                            VT[64 * par:64 * par + 64, b2, :, 0:32],
                            pvt[:].rearrange("k o (h d) -> k (o h) d", h=4))

                # ---- P2: k -> k1 ----
                xk = load_x(pkp, 'k')
                x1k = pkp.tile([128, 2, 8, 8, BL], FP8, name="x1k")
                wk = load_weights(nc, pkp, dt["wk"].ap(), tag="k",
                                  resident=True)
                for oh in range(2):
                    ps = conv_oh(nc, psp, wk, [xk], oh, f"p2{oh}",
                                 [f"cvS{oh}"])
                    relu_out(ps[0], x1k, oh, bias_t['k'])

                # ---- P3 oh=0 -> attn heads 0-3 -> P3 oh=1 -> attn 4-7 ----
                with tc.tile_pool(name="atps", bufs=1, space="PSUM") as atp:
                    for oh in range(2):
                        ps = conv_oh(nc, psp, wk, [x1k, x1q], oh, f"p3{oh}",
                                     ["cvS0", "cvS1"])
                        final_out(ps[0], hh['k'], oh, bias_t['k'])
                        final_out(ps[1], hh['q'], oh, bias_t['k'])
                        attn_half(atp, oh)

            # ---- out transposes + output projection (conv psum freed) ----
            with tc.tile_pool(name="tlps", bufs=1, space="PSUM") as tlp:
                for b in range(BL):
                    pot = tlp.tile([128, 2, 64], BF16, tag="pot",
                                   name=f"pot{b}", bufs=2)
                    for oh in range(2):
                        nc.tensor.transpose(pot[:, oh, :], OA[:, b, oh, :],
                                            ident_t[:64, :64])
                    nc.vector.tensor_copy(concat[:, :, b, :], pot[:])
                for blk in range(8):
                    pspr = tlp.tile([128, 256], F32, tag="pspr",
                                    name=f"pspr{blk}", bufs=2)
                    for oh in range(2):
                        cslice = concat[:, oh].rearrange("c b p -> c (b p)")
                        nc.tensor.matmul(
                            pspr[:], cslice[:, blk * 128:(blk + 1) * 128],
                            wo_tt[oh][:], start=(oh == 0), stop=False)
                    nc.tensor.matmul(pspr[:], ones_t[:], bo_t[:],
                                     start=False, stop=True)
                    nc.scalar.activation(
                        out_sb[:, blk, :], pspr[:],
                        mybir.ActivationFunctionType.Identity)
                    nc.sync.dma_start(dt["out"][blk], out_sb[:, blk, :])
    dedupe_ldweights(nc)
    nc.compile()
    return nc


def prep_static(wk, bk, wq, bq, wv, bv, wo, bo):
    st = {}
    for nmm, w, b in (("q", wq, bq), ("k", wk, bk), ("v", wv, bv)):
        w = np.asarray(w, np.float32)
        wt = np.empty((128, NT, 2, 256), np.float32)
        for t, (sr, sc, *_r) in enumerate(TAPS):
            wtap = w[:, :, sr + 7, sc + 7] * WSCALE     # [o, c]
            wt[:, t, :, :] = wtap.T.reshape(2, 128, 256).transpose(1, 0, 2)
        st[f"w{nmm}"] = wt.astype(E4M3)
        st[f"bias{nmm}"] = np.ascontiguousarray(
            np.asarray(b, np.float32).reshape(2, 128))
    st["wo_t"] = np.ascontiguousarray(
        np.asarray(wo, np.float32).T).reshape(2, 128, 256).astype(
        ml_dtypes.bfloat16)
    st["bo"] = np.asarray(bo, np.float32).reshape(1, 256).astype(
        ml_dtypes.bfloat16)
    st["ones"] = np.ones((1, 128), ml_dtypes.bfloat16)
    st["ident"] = np.eye(128, dtype=ml_dtypes.bfloat16)
    return st


def prep_core_x(x, core):
    xs = np.asarray(x[core * BL:(core + 1) * BL], np.float32)
    xt = xs.transpose(3, 1, 2, 0).reshape(2, 128, 8, 8, BL)
    return np.ascontiguousarray(xt.transpose(1, 0, 2, 3, 4)).astype(E4M3)


def make_in_maps(q, k, v, st):
    in_maps = []
    for core in range(NCORES):
        m = dict(st)
        m["xq"] = prep_core_x(q, core)
        m["xk"] = prep_core_x(k, core)
        m["xv"] = prep_core_x(v, core)
        in_maps.append(m)
    return in_maps


def gather_out(results):
    outs = [r["out"].reshape(BL, 8, 8, D) for r in results]
    return np.concatenate(outs, axis=0)


_NC_CACHE = None


def _get_nc():
    global _NC_CACHE
    if _NC_CACHE is None:
        _NC_CACHE = build_kernel()
    return _NC_CACHE


def kernel(q, k, v, wk, bk, wq, bq, wv, bv, wo, bo):
    nc = _get_nc()
    st = prep_static(wk, bk, wq, bq, wv, bv, wo, bo)
    in_maps = make_in_maps(np.asarray(q), np.asarray(k), np.asarray(v), st)
    res = bass_utils.run_bass_kernel_spmd(
        nc, in_maps, core_ids=list(range(NCORES)))
    return gather_out(res.results)
